# revision 1
# baseline (speedup 1.0000x reference)
"""AttentionSubsample on 8 Trainium2 NeuronCores.

Strategy: data-parallel over batch B (64 -> 8 per core). Weights and the
relative-position bias table are replicated. The batch-norm layers use
training-mode statistics over the FULL batch, so the per-channel mean /
variance reductions cross cores; they are expressed as global-axis means
that the partitioner lowers to all-reduces over the 8-core mesh. Attention
(QK^T, softmax with gathered rel-pos bias, AV), the hard-swish and all
three linear+BN layers run on-device; the host only shards inputs and
gathers the per-core output shards.
"""

import numpy as np
import jax
import jax.numpy as jnp
from jax.sharding import Mesh, PartitionSpec as P, NamedSharding

RES, RES_, STRIDE = 28, 14, 2
H, KD, D = 16, 32, 64
EPS = 1e-5
N_CORES = 8


def _linear_bn(x, W, g, b):
    y = jnp.einsum("bnc,oc->bno", x, W)
    m = y.mean(axis=(0, 1))
    v = (y * y).mean(axis=(0, 1)) - m * m  # biased var, batch-global
    return (y - m) * (g / jnp.sqrt(v + EPS)) + b


def _model(x, W_kv, g_kv, b_kv, W_q, g_q, b_q, W_proj, g_proj, b_proj,
           attn_biases, bias_idxs):
    B, N, C = x.shape
    scale = KD ** -0.5
    kv = _linear_bn(x, W_kv, g_kv, b_kv).reshape(B, N, H, KD + D)
    k, v = kv[..., :KD], kv[..., KD:]
    xq = x.reshape(B, RES, RES, C)[:, ::STRIDE, ::STRIDE].reshape(
        B, RES_ * RES_, C)
    q = _linear_bn(xq, W_q, g_q, b_q).reshape(B, RES_ * RES_, H, KD)
    bias = attn_biases[:, bias_idxs]
    attn = jnp.einsum("bqhd,bkhd->bhqk", q, k) * scale + bias
    attn = jax.nn.softmax(attn, axis=-1)
    out = jnp.einsum("bhqk,bkhd->bqhd", attn, v).reshape(
        B, RES_ * RES_, H * D)
    out = jax.nn.hard_swish(out)
    return _linear_bn(out, W_proj, g_proj, b_proj)


_compiled = None


def _get_fn():
    global _compiled
    if _compiled is None:
        devs = jax.devices()[:N_CORES]
        mesh = Mesh(np.asarray(devs), ("b",))
        sb = NamedSharding(mesh, P("b"))
        rep = NamedSharding(mesh, P())
        in_sh = (sb,) + (rep,) * 11
        _compiled = jax.jit(_model, in_shardings=in_sh, out_shardings=sb)
    return _compiled


def kernel(x, W_kv, g_kv, b_kv, W_q, g_q, b_q, W_proj, g_proj, b_proj,
           attn_biases, bias_idxs):
    f = _get_fn()
    out = f(jnp.asarray(x), jnp.asarray(W_kv), jnp.asarray(g_kv),
            jnp.asarray(b_kv), jnp.asarray(W_q), jnp.asarray(g_q),
            jnp.asarray(b_q), jnp.asarray(W_proj), jnp.asarray(g_proj),
            jnp.asarray(b_proj), jnp.asarray(attn_biases),
            jnp.asarray(bias_idxs))
    return np.asarray(out)


# revision 2
# speedup vs baseline: 84.0620x; 84.0620x over previous
"""AttentionSubsample on 8 Trainium2 NeuronCores.

Strategy: data-parallel over batch B (64 -> 8 per core). Weights and the
relative-position bias table are replicated. The batch-norm layers use
training-mode statistics over the FULL batch, so the per-channel mean /
variance reductions cross cores; they are expressed as global-axis means
that the partitioner lowers to all-reduces over the 8-core mesh. Attention
(QK^T, softmax with gathered rel-pos bias, AV), the hard-swish and all
three linear+BN layers run on-device; the host only shards inputs and
gathers the per-core output shards.

Matmul operands are cast to bf16 (fp32 accumulation) — the TensorE runs
bf16 at 4x the fp32 rate and the end-to-end relative error stays ~1e-3.
BN statistics and softmax stay in fp32.
"""

import numpy as np
import jax
import jax.numpy as jnp
from jax.sharding import Mesh, PartitionSpec as P, NamedSharding

RES, RES_, STRIDE = 28, 14, 2
H, KD, D = 16, 32, 64
EPS = 1e-5
N_CORES = 8
BF = jnp.bfloat16
F32 = jnp.float32


def _mm(a, b, spec):
    return jnp.einsum(spec, a.astype(BF), b.astype(BF),
                      preferred_element_type=F32)


def _linear_bn(x, W, g, b):
    y = _mm(x, W, "bnc,oc->bno")
    m = y.mean(axis=(0, 1))
    v = (y * y).mean(axis=(0, 1)) - m * m  # biased var, batch-global
    return (y - m) * (g / jnp.sqrt(v + EPS)) + b


def _model(x, W_kv, g_kv, b_kv, W_q, g_q, b_q, W_proj, g_proj, b_proj,
           attn_biases, bias_idxs):
    B, N, C = x.shape
    scale = KD ** -0.5
    kv = _linear_bn(x, W_kv, g_kv, b_kv).reshape(B, N, H, KD + D)
    k, v = kv[..., :KD], kv[..., KD:]
    xq = x.reshape(B, RES, RES, C)[:, ::STRIDE, ::STRIDE].reshape(
        B, RES_ * RES_, C)
    q = _linear_bn(xq, W_q, g_q, b_q).reshape(B, RES_ * RES_, H, KD)
    bias = attn_biases[:, bias_idxs]
    attn = _mm(q, k, "bqhd,bkhd->bhqk") * scale + bias
    attn = jax.nn.softmax(attn, axis=-1)
    out = _mm(attn, v, "bhqk,bkhd->bqhd").reshape(B, RES_ * RES_, H * D)
    out = jax.nn.hard_swish(out)
    return _linear_bn(out, W_proj, g_proj, b_proj)


_state = None

_ARG_NAMES = ("x", "W_kv", "g_kv", "b_kv", "W_q", "g_q", "b_q", "W_proj",
              "g_proj", "b_proj", "attn_biases", "bias_idxs")


def _get_state():
    global _state
    if _state is None:
        devs = jax.devices()[:N_CORES]
        mesh = Mesh(np.asarray(devs), ("b",))
        sb = NamedSharding(mesh, P("b"))
        rep = NamedSharding(mesh, P())
        in_sh = (sb,) + (rep,) * 11
        fn = jax.jit(_model, in_shardings=in_sh, out_shardings=sb)
        _state = (fn, in_sh)
    return _state


def _device_args(kw):
    _, in_sh = _get_state()
    return tuple(jax.device_put(jnp.asarray(kw[n]), s)
                 for n, s in zip(_ARG_NAMES, in_sh))


def kernel(**inputs):
    fn, _ = _get_state()
    out = fn(*_device_args(inputs))
    return np.asarray(out)


def run_on_device(dargs):
    """Device-resident args -> device output (for device-time measurement)."""
    fn, _ = _get_state()
    return fn(*dargs)


# revision 3
# speedup vs baseline: 317.7243x; 3.7796x over previous
"""AttentionSubsample on 8 Trainium2 NeuronCores.

Strategy: data-parallel over batch B (64 -> 8 per core). Weights and the
relative-position bias table are replicated. The batch-norm layers use
training-mode statistics over the FULL batch, so the per-channel mean /
variance reductions cross cores; they are expressed as global-axis means
that the partitioner lowers to all-reduces over the 8-core mesh. Attention
(QK^T, softmax with gathered rel-pos bias, AV), the hard-swish and all
three linear+BN layers run on-device; the host only shards inputs and
gathers the per-core output shards.

Matmul operands are cast to bf16 (fp32 accumulation) — the TensorE runs
bf16 at 4x the fp32 rate and the end-to-end relative error stays ~1e-3.
BN statistics and softmax stay in fp32.
"""

import numpy as np
import jax
import jax.numpy as jnp
from jax.sharding import Mesh, PartitionSpec as P, NamedSharding

RES, RES_, STRIDE = 28, 14, 2
H, KD, D = 16, 32, 64
EPS = 1e-5
N_CORES = 8
BF = jnp.bfloat16
F32 = jnp.float32


def _mm(a, b, spec):
    return jnp.einsum(spec, a.astype(BF), b.astype(BF),
                      preferred_element_type=F32)


def _linear_bn(x, W, g, b):
    y = _mm(x, W, "bnc,oc->bno")
    m = y.mean(axis=(0, 1))
    v = (y * y).mean(axis=(0, 1)) - m * m  # biased var, batch-global
    return (y - m) * (g / jnp.sqrt(v + EPS)) + b


def _model(x, W_kv, g_kv, b_kv, W_q, g_q, b_q, W_proj, g_proj, b_proj,
           attn_biases, bias_idxs):
    B, N, C = x.shape
    scale = KD ** -0.5
    kv = _linear_bn(x, W_kv, g_kv, b_kv).reshape(B, N, H, KD + D)
    # head-major [B,H,tok,dim] so attention lowers to plain batched matmuls
    kv = kv.transpose(0, 2, 1, 3)
    k, v = kv[..., :KD], kv[..., KD:]
    xq = x.reshape(B, RES, RES, C)[:, ::STRIDE, ::STRIDE].reshape(
        B, RES_ * RES_, C)
    q = _linear_bn(xq, W_q, g_q, b_q).reshape(B, RES_ * RES_, H, KD)
    q = q.transpose(0, 2, 1, 3)
    bias = attn_biases[:, bias_idxs]
    attn = _mm(q, k, "bhqd,bhkd->bhqk") * scale + bias
    attn = jax.nn.softmax(attn, axis=-1)
    out = _mm(attn, v, "bhqk,bhkd->bhqd")
    out = out.transpose(0, 2, 1, 3).reshape(B, RES_ * RES_, H * D)
    out = jax.nn.hard_swish(out)
    return _linear_bn(out, W_proj, g_proj, b_proj)


_state = None

_ARG_NAMES = ("x", "W_kv", "g_kv", "b_kv", "W_q", "g_q", "b_q", "W_proj",
              "g_proj", "b_proj", "attn_biases", "bias_idxs")


def _get_state():
    global _state
    if _state is None:
        devs = jax.devices()[:N_CORES]
        mesh = Mesh(np.asarray(devs), ("b",))
        sb = NamedSharding(mesh, P("b"))
        rep = NamedSharding(mesh, P())
        in_sh = (sb,) + (rep,) * 11
        fn = jax.jit(_model, in_shardings=in_sh, out_shardings=sb)
        _state = (fn, in_sh)
    return _state


def _device_args(kw):
    _, in_sh = _get_state()
    return tuple(jax.device_put(jnp.asarray(kw[n]), s)
                 for n, s in zip(_ARG_NAMES, in_sh))


def kernel(**inputs):
    fn, _ = _get_state()
    out = fn(*_device_args(inputs))
    return np.asarray(out)


def run_on_device(dargs):
    """Device-resident args -> device output (for device-time measurement)."""
    fn, _ = _get_state()
    return fn(*dargs)
